# revision 17
# baseline (speedup 1.0000x reference)
"""Capsule dynamic-routing kernel for Trainium2 (8 NeuronCores, data-parallel over batch).

Reformulation that never materializes u_hat = u_vecs @ W.T ([B, 1024, 2048]):
with s[n, :] = sum_i c[n, i] * u_vecs[i, :]              (32x256 per batch)
  o[n, d]  = sum_d' W[n*64+d, d'] * s[n, d']             (block-diag, tiny)
  g[n, d'] = sum_d W[n*64+d, d'] * o_hat[n, d]           (block-diag, tiny)
  b[n, i]  = sum_d' u_vecs[i, d'] * g[n, d']             (1024x32 per batch)
This is exact (linearity) and cuts compute ~80x vs materializing u_hat.

All PE products run as fp16 hi/lo 3-term splits (x*y ~ xh*yh + xh*yl + xl*yh,
residual ~2^-22) so matmuls run at 1 cycle/row with fast weight loads while
keeping fp32-equivalent accuracy. Batches are processed in 2 groups of 4 so
one group's vector-engine chain (softmax, norms, splits) overlaps the other
group's PE phases. Host pre-tiles all inputs into partition-major layouts so
every DMA is contiguous per partition.
"""
import sys

sys.path.insert(0, "/opt/trn_rl_repo")

import numpy as np

B, IN, ID = 64, 1024, 256
NC, DC = 32, 64
ROUTINGS = 3
NCORES = 8
BPC = B // NCORES      # batches per core (8)
NCH = ID // 128        # d' chunks (2)
NT = IN // 128         # i chunks (8)
NG = 2                 # batch groups per core
GB = BPC // NG         # batches per group (4)

_CACHE = {}


def _build_program():
    import concourse.bass as bass
    import concourse.tile as tile
    from concourse import mybir, bacc

    f32 = mybir.dt.float32
    f16 = mybir.dt.float16
    nc = bacc.Bacc("TRN2", target_bir_lowering=False, debug=False,
                   num_devices=NCORES)

    # all inputs pre-tiled on host: partition dim (128) leads each batch slab
    un_hi_d = nc.dram_tensor("un_hi", [BPC, 128, NT, ID], f16, kind="ExternalInput")
    un_lo_d = nc.dram_tensor("un_lo", [BPC, 128, NT, ID], f16, kind="ExternalInput")
    ut_hi_d = nc.dram_tensor("ut_hi", [BPC, 128, NCH, IN], f16, kind="ExternalInput")
    ut_lo_d = nc.dram_tensor("ut_lo", [BPC, 128, NCH, IN], f16, kind="ExternalInput")
    wot_hi_d = nc.dram_tensor("wot_hi", [128, NCH, NC, DC], f16, kind="ExternalInput")
    wot_lo_d = nc.dram_tensor("wot_lo", [128, NCH, NC, DC], f16, kind="ExternalInput")
    wg_hi_d = nc.dram_tensor("wg_hi", [DC, NC, ID], f16, kind="ExternalInput")
    wg_lo_d = nc.dram_tensor("wg_lo", [DC, NC, ID], f16, kind="ExternalInput")
    out_d = nc.dram_tensor("out", [DC, NC, BPC], f32, kind="ExternalOutput")

    with tile.TileContext(nc) as tc:
        with tc.tile_pool(name="weights", bufs=1) as wpool, \
             tc.tile_pool(name="udata", bufs=1) as upool, \
             tc.tile_pool(name="ut_stream", bufs=3) as utpool, \
             tc.tile_pool(name="state", bufs=1) as spool, \
             tc.tile_pool(name="small", bufs=2) as mpool, \
             tc.tile_pool(name="ps_big", bufs=1, space="PSUM") as psb, \
             tc.tile_pool(name="ps_bst", bufs=3, space="PSUM") as psbst, \
             tc.tile_pool(name="ps_small", bufs=1, space="PSUM") as pss:

            # ---- u_vecs first (the s-product consumes batch 0 immediately) ----
            un_hi = upool.tile([128, BPC, NT, ID], f16, tag="un_hi")
            un_lo = upool.tile([128, BPC, NT, ID], f16, tag="un_lo")
            for b in range(BPC):
                nc.sync.dma_start(out=un_hi[:, b, :, :], in_=un_hi_d[b])
                nc.sync.dma_start(out=un_lo[:, b, :, :], in_=un_lo_d[b])

            # ---- constants / weights (resident whole kernel) ----
            wot_hi = wpool.tile([128, NCH, NC, DC], f16, tag="wot_hi")
            wot_lo = wpool.tile([128, NCH, NC, DC], f16, tag="wot_lo")
            nc.sync.dma_start(out=wot_hi, in_=wot_hi_d[:, :, :, :])
            nc.sync.dma_start(out=wot_lo, in_=wot_lo_d[:, :, :, :])
            wg_hi = wpool.tile([DC, NC, ID], f16, tag="wg_hi")
            wg_lo = wpool.tile([DC, NC, ID], f16, tag="wg_lo")
            nc.sync.dma_start(out=wg_hi, in_=wg_hi_d[:, :, :])
            nc.sync.dma_start(out=wg_lo, in_=wg_lo_d[:, :, :])
            c0 = wpool.tile([128, NC], f16, tag="c0")   # 1/32 exact in fp16
            nc.vector.memset(c0, 1.0 / NC)
            ones_col = wpool.tile([DC, 1], f32, tag="ones_col")
            nc.vector.memset(ones_col, 1.0)
            ones_row = wpool.tile([1, 128], f32, tag="ones_row")
            nc.vector.memset(ones_row, 1.0)
            eps7 = wpool.tile([1, 1], f32, tag="eps7")
            nc.vector.memset(eps7, 1e-7)


            # per-group state, indexed [g]
            c_hi = [None] * NG
            c_lo = [None] * NG
            o_sb = [None] * NG
            nsq = [None] * NG

            for it in range(ROUTINGS):
                sTh = [None] * NG
                sTl = [None] * NG
                sTs = mpool.tile([128, NCH, BPC, NC], f32, tag="sTs",
                                 name=f"sTs_{it}")
                sT_hi = mpool.tile([128, NCH, BPC, NC], f16, tag="sT_hi",
                                   name=f"sT_hi_{it}")
                sT_lo = mpool.tile([128, NCH, BPC, NC], f16, tag="sT_lo",
                                   name=f"sT_lo_{it}")
                for g in range(NG):
                    # ---- s-product for group g
                    sT_ps = psb.tile([128, NCH, GB, NC], f32, tag=f"proj{g}")
                    for bl in range(GB):
                        b = g * GB + bl
                        for h in range(NCH):
                            if it == 0:
                                terms = [(un_hi, None), (un_lo, None)]
                            else:
                                terms = [(un_hi, c_hi[g]), (un_lo, c_hi[g]),
                                         (un_hi, c_lo[g])]
                            nterm = len(terms)
                            for t in range(NT):
                                for k, (u_t_, c_t_) in enumerate(terms):
                                    rhs = c0 if it == 0 else c_t_[:, bl * NT + t, :]
                                    nc.tensor.matmul(
                                        sT_ps[:, h, bl, :],
                                        u_t_[:, b, t, h * 128:(h + 1) * 128],
                                        rhs,
                                        start=(t == 0 and k == 0),
                                        stop=(t == NT - 1 and k == nterm - 1))
                    # split sT -> fp16 hi/lo (o-product rhs), into full-batch tiles
                    gsl = slice(g * GB, (g + 1) * GB)
                    nc.vector.tensor_copy(sTs[:, :, gsl, :], sT_ps)
                    nc.vector.tensor_copy(sT_hi[:, :, gsl, :], sTs[:, :, gsl, :])
                    nc.vector.tensor_tensor(out=sT_lo[:, :, gsl, :],
                                            in0=sTs[:, :, gsl, :],
                                            in1=sT_hi[:, :, gsl, :],
                                            op=mybir.AluOpType.subtract)

                # ---- o-product, all batches (N=8)
                o_ps = pss.tile([DC, NC, BPC], f32, tag="o")
                for n in range(NC):
                    terms = [(wot_hi, sT_hi), (wot_hi, sT_lo), (wot_lo, sT_hi)]
                    for h in range(NCH):
                        for k, (w_t_, s_t_) in enumerate(terms):
                            nc.tensor.matmul(
                                o_ps[:, n, :],
                                w_t_[:, h, n, :],
                                s_t_[:, h, :, n],
                                start=(h == 0 and k == 0),
                                stop=(h == NCH - 1 and k == 2))
                o_sbf = mpool.tile([DC, NC, BPC], f32, tag="o_sb", name=f"o_sb_{it}")
                nc.vector.tensor_copy(o_sbf, o_ps)

                if it < ROUTINGS - 1:
                    # split o (unnormalized) for g-product rhs -- FIRST so the
                    # g-product matmuls enter the PE stream right after the
                    # o-product; the norm chain below overlaps them.
                    o_hi = mpool.tile([DC, NC, BPC], f16, tag="o_hi")
                    nc.vector.tensor_copy(o_hi, o_sbf)
                    o_lo = mpool.tile([DC, NC, BPC], f16, tag="o_lo")
                    nc.vector.tensor_tensor(out=o_lo, in0=o_sbf, in1=o_hi,
                                            op=mybir.AluOpType.subtract)

                    # ---- g-product (unnormalized), all batches
                    g_ps = psb.tile([128, NCH, NC, BPC], f32, tag="proj0")
                    for n in range(NC):
                        terms = [(wg_hi, o_hi), (wg_hi, o_lo), (wg_lo, o_hi)]
                        for h in range(NCH):
                            for k, (w_t_, o_t_) in enumerate(terms):
                                nc.tensor.matmul(
                                    g_ps[:, h, n, :],
                                    w_t_[:, n, h * 128:(h + 1) * 128],
                                    o_t_[:, n, :],
                                    start=(k == 0), stop=(k == 2))

                    # ---- norms (runs on DVE/ACT while the PE does g-product)
                    osq = mpool.tile([DC, NC * BPC], f32, tag="osq")
                    nc.vector.tensor_tensor(
                        out=osq, in0=o_sbf.rearrange("p n b -> p (n b)"),
                        in1=o_sbf.rearrange("p n b -> p (n b)"),
                        op=mybir.AluOpType.mult)
                    nsq_ps = pss.tile([1, NC * BPC], f32, tag="aux")
                    nc.tensor.matmul(nsq_ps, ones_col, osq, start=True, stop=True)
                    # r = 1 / max(sqrt(nsq), 1e-12)  (F.normalize); ACT reads PSUM
                    rt = mpool.tile([1, NC * BPC], f32, tag="rt")
                    nc.scalar.activation(rt, nsq_ps,
                                         mybir.ActivationFunctionType.Sqrt)
                    nc.vector.tensor_scalar_max(rt, rt, 1e-12)
                    rr = mpool.tile([1, NC * BPC], f32, tag="rr")
                    nc.vector.reciprocal(rr, rt)
                    rrep_ps = pss.tile([128, NC * BPC], f32, tag="aux2")
                    nc.tensor.matmul(rrep_ps, ones_row, rr, start=True, stop=True)
                    rrep = mpool.tile([128, NC * BPC], f32, tag="rrep")
                    nc.vector.tensor_copy(rrep, rrep_ps)

                    # normalize folded in via r; split to fp16 hi/lo
                    gs = mpool.tile([128, NCH, NC, BPC], f32, tag="gs")
                    rrep_b = bass.AP(
                        tensor=rrep.tensor, offset=rrep.offset,
                        ap=[rrep.ap[0], [0, NCH], [BPC, NC], [1, BPC]])
                    nc.vector.tensor_tensor(out=gs, in0=g_ps, in1=rrep_b,
                                            op=mybir.AluOpType.mult)
                    gs_hi = mpool.tile([128, NCH, NC, BPC], f16, tag="gs_hi")
                    nc.vector.tensor_copy(gs_hi, gs)
                    gs_lo = mpool.tile([128, NCH, NC, BPC], f16, tag="gs_lo")
                    nc.vector.tensor_tensor(out=gs_lo, in0=gs, in1=gs_hi,
                                            op=mybir.AluOpType.subtract)

                    for g in range(NG):
                        # ---- b-product (u_t streamed from HBM, prefetched)
                        # per-batch psum tile (1 bank) + per-batch exp
                        E = spool.tile([128, GB * NT, NC], f32, tag=f"E{g}")
                        for bl in range(GB):
                            b = g * GB + bl
                            bst_ps = psbst.tile([128, NT, NC], f32, tag="bst")
                            uth = utpool.tile([128, NCH, IN], f16, tag="uth")
                            nc.gpsimd.dma_start(out=uth, in_=ut_hi_d[b])
                            utl = utpool.tile([128, NCH, IN], f16, tag="utl")
                            nc.gpsimd.dma_start(out=utl, in_=ut_lo_d[b])
                            for t in range(NT):
                                terms = [(uth, gs_hi), (uth, gs_lo), (utl, gs_hi)]
                                nterm = len(terms)
                                for h in range(NCH):
                                    for k, (u_t_, g_t_) in enumerate(terms):
                                        nc.tensor.matmul(
                                            bst_ps[:, t, :],
                                            u_t_[:, h, t * 128:(t + 1) * 128],
                                            g_t_[:, h, :, b],
                                            start=(h == 0 and k == 0),
                                            stop=(h == NCH - 1 and k == nterm - 1))
                            nc.scalar.activation(E[:, bl * NT:(bl + 1) * NT, :], bst_ps,
                                                 mybir.ActivationFunctionType.Exp)

                        # ---- softmax over n (free axis)
                        z = mpool.tile([128, GB * NT], f32, tag=f"z{g}")
                        nc.vector.reduce_sum(z, E, axis=mybir.AxisListType.X)
                        zr = mpool.tile([128, GB * NT], f32, tag=f"zr{g}")
                        nc.vector.reciprocal(zr, z)
                        c_f = spool.tile([128, GB * NT, NC], f32, tag=f"c_f{g}")
                        zr_b = bass.AP(tensor=zr.tensor, offset=zr.offset,
                                       ap=[zr.ap[0], zr.ap[1], [0, NC]])
                        nc.vector.tensor_tensor(out=c_f, in0=E, in1=zr_b,
                                                op=mybir.AluOpType.mult)
                        c_hi[g] = spool.tile([128, GB * NT, NC], f16, tag=f"c_hi{g}", name=f"c_hi{g}_{it}")
                        nc.vector.tensor_copy(c_hi[g], c_f)
                        c_lo[g] = spool.tile([128, GB * NT, NC], f16, tag=f"c_lo{g}", name=f"c_lo{g}_{it}")
                        nc.vector.tensor_tensor(out=c_lo[g], in0=c_f, in1=c_hi[g],
                                                op=mybir.AluOpType.subtract)
                else:
                    # ---- squash: out = sqrt(sq)/(0.5+sq) * o, sq = nsq + 1e-7
                    osq = mpool.tile([DC, NC * BPC], f32, tag="osq")
                    nc.vector.tensor_tensor(
                        out=osq, in0=o_sbf.rearrange("p n b -> p (n b)"),
                        in1=o_sbf.rearrange("p n b -> p (n b)"),
                        op=mybir.AluOpType.mult)
                    nsq_ps = pss.tile([1, NC * BPC], f32, tag="aux")
                    nc.tensor.matmul(nsq_ps, ones_col, osq, start=True, stop=True)
                    nsqf = mpool.tile([1, NC * BPC], f32, tag="nsq_sb")
                    nc.vector.tensor_copy(nsqf, nsq_ps)
                    rt = mpool.tile([1, NC * BPC], f32, tag="rt")
                    nc.scalar.activation(rt, nsqf,
                                         mybir.ActivationFunctionType.Sqrt,
                                         bias=eps7)
                    den = mpool.tile([1, NC * BPC], f32, tag="den")
                    nc.vector.tensor_scalar_add(den, nsqf, 0.5 + 1e-7)
                    dr = mpool.tile([1, NC * BPC], f32, tag="dr")
                    nc.vector.reciprocal(dr, den)
                    tsc = mpool.tile([1, NC * BPC], f32, tag="tsc")
                    nc.vector.tensor_tensor(out=tsc, in0=rt, in1=dr,
                                            op=mybir.AluOpType.mult)
                    trep_ps = pss.tile([DC, NC * BPC], f32, tag="aux")
                    nc.tensor.matmul(trep_ps, ones_row[:, :DC], tsc,
                                     start=True, stop=True)
                    trep = mpool.tile([DC, NC * BPC], f32, tag="trep")
                    nc.vector.tensor_copy(trep, trep_ps)
                    of = mpool.tile([DC, NC, BPC], f32, tag="of")
                    nc.vector.tensor_tensor(
                        out=of.rearrange("p n b -> p (n b)"),
                        in0=o_sbf.rearrange("p n b -> p (n b)"),
                        in1=trep, op=mybir.AluOpType.mult)
                    nc.sync.dma_start(out=out_d[:, :, :], in_=of)
    nc.compile()
    return nc


def get_program():
    if "nc" not in _CACHE:
        _CACHE["nc"] = _build_program()
    return _CACHE["nc"]


def _split16(x):
    hi = x.astype(np.float16)
    lo = (x - hi.astype(np.float32)).astype(np.float16)
    return hi, lo


def make_in_maps(u_vecs, W):
    u_vecs = np.ascontiguousarray(u_vecs, dtype=np.float32)
    W = np.ascontiguousarray(W, dtype=np.float32)
    wot = np.ascontiguousarray(W.reshape(NC, DC, ID).transpose(2, 0, 1))  # [d', n, d]
    wg = np.ascontiguousarray(W.reshape(NC, DC, ID).transpose(1, 0, 2))   # [d, n, d']
    wot_hi, wot_lo = _split16(wot)
    # pre-tile wot to [128, NCH, NC, DC]
    wot_hi = np.ascontiguousarray(wot_hi.reshape(NCH, 128, NC, DC).transpose(1, 0, 2, 3))
    wot_lo = np.ascontiguousarray(wot_lo.reshape(NCH, 128, NC, DC).transpose(1, 0, 2, 3))
    wg_hi, wg_lo = _split16(wg)

    u4 = u_vecs.reshape(NCORES, BPC, IN, ID)
    un_hi, un_lo = _split16(u4)
    # u_nat pre-tiled: [BPC, IN, ID] -> [BPC, 128, NT, ID]
    def tile_nat(x):
        return np.ascontiguousarray(
            x.reshape(NCORES, BPC, NT, 128, ID).transpose(0, 1, 3, 2, 4))
    # u_t pre-tiled: [BPC, ID, IN] -> [BPC, 128, NCH, IN]
    def tile_t(x):
        xt = x.transpose(0, 1, 3, 2)  # [NCORES, BPC, ID, IN]
        return np.ascontiguousarray(
            xt.reshape(NCORES, BPC, NCH, 128, IN).transpose(0, 1, 3, 2, 4))
    un_hi_t, un_lo_t = tile_nat(un_hi), tile_nat(un_lo)
    ut_hi_t, ut_lo_t = tile_t(un_hi), tile_t(un_lo)
    return [
        {"un_hi": un_hi_t[k], "un_lo": un_lo_t[k],
         "ut_hi": ut_hi_t[k], "ut_lo": ut_lo_t[k],
         "wot_hi": wot_hi, "wot_lo": wot_lo,
         "wg_hi": wg_hi, "wg_lo": wg_lo}
        for k in range(NCORES)
    ]


def kernel(u_vecs: np.ndarray, W: np.ndarray) -> np.ndarray:
    from concourse.bass_utils import run_bass_kernel_spmd

    nc = get_program()
    in_maps = make_in_maps(u_vecs, W)
    res = run_bass_kernel_spmd(nc, in_maps, list(range(NCORES))).results
    # per-core out: [DC, NC, BPC] -> [BPC, NC, DC]; stack cores -> [B, NC, DC]
    parts = [res[k]["out"].transpose(2, 1, 0) for k in range(NCORES)]
    return np.ascontiguousarray(np.concatenate(parts, axis=0))


# revision 18
# speedup vs baseline: 1.1614x; 1.1614x over previous
"""Capsule dynamic-routing kernel for Trainium2 (8 NeuronCores, data-parallel over batch).

Reformulation that never materializes u_hat = u_vecs @ W.T ([B, 1024, 2048]):
with s[n, :] = sum_i c[n, i] * u_vecs[i, :]              (32x256 per batch)
  o[n, d]  = sum_d' W[n*64+d, d'] * s[n, d']             (block-diag, tiny)
  g[n, d'] = sum_d W[n*64+d, d'] * o_hat[n, d]           (block-diag, tiny)
  b[n, i]  = sum_d' u_vecs[i, d'] * g[n, d']             (1024x32 per batch)
This is exact (linearity) and cuts compute ~80x vs materializing u_hat.

All PE products run as fp16 hi/lo 3-term splits (x*y ~ xh*yh + xh*yl + xl*yh,
residual ~2^-22) so matmuls run at 1 cycle/row with fast weight loads while
keeping fp32-equivalent accuracy. Batches are processed in 2 groups of 4 so
one group's vector-engine chain (softmax, norms, splits) overlaps the other
group's PE phases. Host pre-tiles all inputs into partition-major layouts so
every DMA is contiguous per partition.
"""
import sys

sys.path.insert(0, "/opt/trn_rl_repo")

import numpy as np

B, IN, ID = 64, 1024, 256
NC, DC = 32, 64
ROUTINGS = 3
NCORES = 8
BPC = B // NCORES      # batches per core (8)
NCH = ID // 128        # d' chunks (2)
NT = IN // 128         # i chunks (8)
NG = 2                 # batch groups per core
GB = BPC // NG         # batches per group (4)

_CACHE = {}


def _build_program():
    import concourse.bass as bass
    import concourse.tile as tile
    from concourse import mybir, bacc

    f32 = mybir.dt.float32
    f16 = mybir.dt.float16
    nc = bacc.Bacc("TRN2", target_bir_lowering=False, debug=False,
                   num_devices=NCORES)

    # all inputs pre-tiled on host: partition dim (128) leads each batch slab
    un_hi_d = nc.dram_tensor("un_hi", [BPC, 128, NT, ID], f16, kind="ExternalInput")
    un_lo_d = nc.dram_tensor("un_lo", [BPC, 128, NT, ID], f16, kind="ExternalInput")
    ut_hi_d = nc.dram_tensor("ut_hi", [BPC, 128, NCH, IN], f16, kind="ExternalInput")
    ut_lo_d = nc.dram_tensor("ut_lo", [BPC, 128, NCH, IN], f16, kind="ExternalInput")
    wot_hi_d = nc.dram_tensor("wot_hi", [128, NCH, NC, DC], f16, kind="ExternalInput")
    wot_lo_d = nc.dram_tensor("wot_lo", [128, NCH, NC, DC], f16, kind="ExternalInput")
    wg_hi_d = nc.dram_tensor("wg_hi", [DC, NC, ID], f16, kind="ExternalInput")
    wg_lo_d = nc.dram_tensor("wg_lo", [DC, NC, ID], f16, kind="ExternalInput")
    out_d = nc.dram_tensor("out", [DC, NC, BPC], f32, kind="ExternalOutput")

    with tile.TileContext(nc) as tc:
        with tc.tile_pool(name="weights", bufs=1) as wpool, \
             tc.tile_pool(name="udata", bufs=1) as upool, \
             tc.tile_pool(name="ut_stream", bufs=3) as utpool, \
             tc.tile_pool(name="state", bufs=1) as spool, \
             tc.tile_pool(name="small", bufs=2) as mpool, \
             tc.tile_pool(name="ps_big", bufs=1, space="PSUM") as psb, \
             tc.tile_pool(name="ps_bst", bufs=3, space="PSUM") as psbst, \
             tc.tile_pool(name="ps_small", bufs=1, space="PSUM") as pss:

            # ---- u_vecs first (the s-product consumes batch 0 immediately) ----
            un_hi = upool.tile([128, BPC, NT, ID], f16, tag="un_hi")
            un_lo = upool.tile([128, BPC, NT, ID], f16, tag="un_lo")
            for b in range(BPC):
                nc.sync.dma_start(out=un_hi[:, b, :, :], in_=un_hi_d[b])
                nc.sync.dma_start(out=un_lo[:, b, :, :], in_=un_lo_d[b])

            # ---- constants / weights (resident whole kernel) ----
            wot_hi = wpool.tile([128, NCH, NC, DC], f16, tag="wot_hi")
            wot_lo = wpool.tile([128, NCH, NC, DC], f16, tag="wot_lo")
            nc.sync.dma_start(out=wot_hi, in_=wot_hi_d[:, :, :, :])
            nc.sync.dma_start(out=wot_lo, in_=wot_lo_d[:, :, :, :])
            wg_hi = wpool.tile([DC, NC, ID], f16, tag="wg_hi")
            wg_lo = wpool.tile([DC, NC, ID], f16, tag="wg_lo")
            nc.sync.dma_start(out=wg_hi, in_=wg_hi_d[:, :, :])
            nc.sync.dma_start(out=wg_lo, in_=wg_lo_d[:, :, :])
            c0 = wpool.tile([128, NC], f16, tag="c0")   # 1/32 exact in fp16
            nc.vector.memset(c0, 1.0 / NC)
            ones_col = wpool.tile([DC, 1], f32, tag="ones_col")
            nc.vector.memset(ones_col, 1.0)
            ones_row = wpool.tile([1, 128], f32, tag="ones_row")
            nc.vector.memset(ones_row, 1.0)
            eps7 = wpool.tile([1, 1], f32, tag="eps7")
            nc.vector.memset(eps7, 1e-7)


            # per-group state, indexed [g]
            c_hi = [None] * NG
            c_lo = [None] * NG
            o_sb = [None] * NG
            nsq = [None] * NG

            for it in range(ROUTINGS):
                sTh = [None] * NG
                sTl = [None] * NG
                sTs = mpool.tile([128, NCH, BPC, NC], f32, tag="sTs",
                                 name=f"sTs_{it}")
                sT_hi = mpool.tile([128, NCH, BPC, NC], f16, tag="sT_hi",
                                   name=f"sT_hi_{it}")
                sT_lo = mpool.tile([128, NCH, BPC, NC], f16, tag="sT_lo",
                                   name=f"sT_lo_{it}")
                for g in range(NG):
                    # ---- s-product for group g
                    sT_ps = psb.tile([128, NCH, GB, NC], f32, tag=f"proj{g}")
                    for bl in range(GB):
                        b = g * GB + bl
                        for h in range(NCH):
                            if it == 0:
                                terms = [(un_hi, None), (un_lo, None)]
                            else:
                                terms = [(un_hi, c_hi[g]), (un_lo, c_hi[g]),
                                         (un_hi, c_lo[g])]
                            nterm = len(terms)
                            for t in range(NT):
                                for k, (u_t_, c_t_) in enumerate(terms):
                                    rhs = c0 if it == 0 else c_t_[:, bl * NT + t, :]
                                    nc.tensor.matmul(
                                        sT_ps[:, h, bl, :],
                                        u_t_[:, b, t, h * 128:(h + 1) * 128],
                                        rhs,
                                        start=(t == 0 and k == 0),
                                        stop=(t == NT - 1 and k == nterm - 1))
                    # split sT -> fp16 hi/lo (o-product rhs), into full-batch tiles
                    gsl = slice(g * GB, (g + 1) * GB)
                    nc.vector.tensor_copy(sTs[:, :, gsl, :], sT_ps)
                    nc.vector.tensor_copy(sT_hi[:, :, gsl, :], sTs[:, :, gsl, :])
                    nc.vector.tensor_tensor(out=sT_lo[:, :, gsl, :],
                                            in0=sTs[:, :, gsl, :],
                                            in1=sT_hi[:, :, gsl, :],
                                            op=mybir.AluOpType.subtract)

                # ---- o-product, all batches (N=8)
                o_ps = pss.tile([DC, NC, BPC], f32, tag="o")
                for n in range(NC):
                    terms = [(wot_hi, sT_hi), (wot_hi, sT_lo), (wot_lo, sT_hi)]
                    for h in range(NCH):
                        for k, (w_t_, s_t_) in enumerate(terms):
                            nc.tensor.matmul(
                                o_ps[:, n, :],
                                w_t_[:, h, n, :],
                                s_t_[:, h, :, n],
                                start=(h == 0 and k == 0),
                                stop=(h == NCH - 1 and k == 2))
                o_sbf = mpool.tile([DC, NC, BPC], f32, tag="o_sb", name=f"o_sb_{it}")
                nc.vector.tensor_copy(o_sbf, o_ps)

                # ---- squared col-norms over d via ones-matmul
                osq = mpool.tile([DC, NC * BPC], f32, tag="osq")
                nc.vector.tensor_tensor(
                    out=osq, in0=o_sbf.rearrange("p n b -> p (n b)"),
                    in1=o_sbf.rearrange("p n b -> p (n b)"),
                    op=mybir.AluOpType.mult)
                nsq_ps = pss.tile([1, NC * BPC], f32, tag="aux")
                nc.tensor.matmul(nsq_ps, ones_col, osq, start=True, stop=True)
                nsqf = mpool.tile([1, NC * BPC], f32, tag="nsq_sb", name=f"nsq_sb_{it}")
                nc.vector.tensor_copy(nsqf, nsq_ps)

                if it < ROUTINGS - 1:
                    # r = 1 / max(sqrt(nsq), 1e-12)  (F.normalize)
                    rt = mpool.tile([1, NC * BPC], f32, tag="rt")
                    nc.scalar.activation(rt, nsqf,
                                         mybir.ActivationFunctionType.Sqrt)
                    nc.vector.tensor_scalar_max(rt, rt, 1e-12)
                    rr = mpool.tile([1, NC * BPC], f32, tag="rr")
                    nc.vector.reciprocal(rr, rt)
                    rrep_ps = pss.tile([128, NC * BPC], f32, tag="aux")
                    nc.tensor.matmul(rrep_ps, ones_row, rr, start=True, stop=True)
                    rrep = mpool.tile([128, NC * BPC], f32, tag="rrep")
                    nc.vector.tensor_copy(rrep, rrep_ps)

                    # split o (unnormalized) for g-product rhs
                    o_hi = mpool.tile([DC, NC, BPC], f16, tag="o_hi")
                    nc.vector.tensor_copy(o_hi, o_sbf)
                    o_lo = mpool.tile([DC, NC, BPC], f16, tag="o_lo")
                    nc.vector.tensor_tensor(out=o_lo, in0=o_sbf, in1=o_hi,
                                            op=mybir.AluOpType.subtract)

                    # ---- g-product (unnormalized), all batches
                    g_ps = psb.tile([128, NCH, NC, BPC], f32, tag="proj0")
                    for n in range(NC):
                        terms = [(wg_hi, o_hi), (wg_hi, o_lo), (wg_lo, o_hi)]
                        for h in range(NCH):
                            for k, (w_t_, o_t_) in enumerate(terms):
                                nc.tensor.matmul(
                                    g_ps[:, h, n, :],
                                    w_t_[:, n, h * 128:(h + 1) * 128],
                                    o_t_[:, n, :],
                                    start=(k == 0), stop=(k == 2))
                    # normalize folded in via r; split to fp16 hi/lo
                    gs = mpool.tile([128, NCH, NC, BPC], f32, tag="gs")
                    rrep_b = bass.AP(
                        tensor=rrep.tensor, offset=rrep.offset,
                        ap=[rrep.ap[0], [0, NCH], [BPC, NC], [1, BPC]])
                    nc.vector.tensor_tensor(out=gs, in0=g_ps, in1=rrep_b,
                                            op=mybir.AluOpType.mult)
                    gs_hi = mpool.tile([128, NCH, NC, BPC], f16, tag="gs_hi")
                    nc.vector.tensor_copy(gs_hi, gs)
                    gs_lo = mpool.tile([128, NCH, NC, BPC], f16, tag="gs_lo")
                    nc.vector.tensor_tensor(out=gs_lo, in0=gs, in1=gs_hi,
                                            op=mybir.AluOpType.subtract)

                    for g in range(NG):
                        # ---- b-product (u_t streamed from HBM, prefetched)
                        # per-batch psum tile (1 bank) + per-batch exp
                        E = spool.tile([128, GB * NT, NC], f32, tag=f"E{g}")
                        for bl in range(GB):
                            b = g * GB + bl
                            bst_ps = psbst.tile([128, NT, NC], f32, tag="bst")
                            uth = utpool.tile([128, NCH, IN], f16, tag="uth")
                            nc.gpsimd.dma_start(out=uth, in_=ut_hi_d[b])
                            utl = utpool.tile([128, NCH, IN], f16, tag="utl")
                            nc.gpsimd.dma_start(out=utl, in_=ut_lo_d[b])
                            for t in range(NT):
                                terms = [(uth, gs_hi), (uth, gs_lo), (utl, gs_hi)]
                                nterm = len(terms)
                                for h in range(NCH):
                                    for k, (u_t_, g_t_) in enumerate(terms):
                                        nc.tensor.matmul(
                                            bst_ps[:, t, :],
                                            u_t_[:, h, t * 128:(t + 1) * 128],
                                            g_t_[:, h, :, b],
                                            start=(h == 0 and k == 0),
                                            stop=(h == NCH - 1 and k == nterm - 1))
                            nc.scalar.activation(E[:, bl * NT:(bl + 1) * NT, :], bst_ps,
                                                 mybir.ActivationFunctionType.Exp)

                        # ---- softmax over n (free axis)
                        z = mpool.tile([128, GB * NT], f32, tag=f"z{g}")
                        nc.vector.reduce_sum(z, E, axis=mybir.AxisListType.X)
                        zr = mpool.tile([128, GB * NT], f32, tag=f"zr{g}")
                        nc.vector.reciprocal(zr, z)
                        c_f = spool.tile([128, GB * NT, NC], f32, tag=f"c_f{g}")
                        zr_b = bass.AP(tensor=zr.tensor, offset=zr.offset,
                                       ap=[zr.ap[0], zr.ap[1], [0, NC]])
                        nc.vector.tensor_tensor(out=c_f, in0=E, in1=zr_b,
                                                op=mybir.AluOpType.mult)
                        c_hi[g] = spool.tile([128, GB * NT, NC], f16, tag=f"c_hi{g}", name=f"c_hi{g}_{it}")
                        nc.vector.tensor_copy(c_hi[g], c_f)
                        c_lo[g] = spool.tile([128, GB * NT, NC], f16, tag=f"c_lo{g}", name=f"c_lo{g}_{it}")
                        nc.vector.tensor_tensor(out=c_lo[g], in0=c_f, in1=c_hi[g],
                                                op=mybir.AluOpType.subtract)
                else:
                    # ---- squash: out = sqrt(sq)/(0.5+sq) * o, sq = nsq + 1e-7
                    rt = mpool.tile([1, NC * BPC], f32, tag="rt")
                    nc.scalar.activation(rt, nsqf,
                                         mybir.ActivationFunctionType.Sqrt,
                                         bias=eps7)
                    den = mpool.tile([1, NC * BPC], f32, tag="den")
                    nc.vector.tensor_scalar_add(den, nsqf, 0.5 + 1e-7)
                    dr = mpool.tile([1, NC * BPC], f32, tag="dr")
                    nc.vector.reciprocal(dr, den)
                    tsc = mpool.tile([1, NC * BPC], f32, tag="tsc")
                    nc.vector.tensor_tensor(out=tsc, in0=rt, in1=dr,
                                            op=mybir.AluOpType.mult)
                    trep_ps = pss.tile([DC, NC * BPC], f32, tag="aux")
                    nc.tensor.matmul(trep_ps, ones_row[:, :DC], tsc,
                                     start=True, stop=True)
                    trep = mpool.tile([DC, NC * BPC], f32, tag="trep")
                    nc.vector.tensor_copy(trep, trep_ps)
                    of = mpool.tile([DC, NC, BPC], f32, tag="of")
                    nc.vector.tensor_tensor(
                        out=of.rearrange("p n b -> p (n b)"),
                        in0=o_sbf.rearrange("p n b -> p (n b)"),
                        in1=trep, op=mybir.AluOpType.mult)
                    nc.sync.dma_start(out=out_d[:, :, :], in_=of)
    nc.compile()
    return nc


def get_program():
    if "nc" not in _CACHE:
        _CACHE["nc"] = _build_program()
    return _CACHE["nc"]


def _split16(x):
    hi = x.astype(np.float16)
    lo = (x - hi.astype(np.float32)).astype(np.float16)
    return hi, lo


def make_in_maps(u_vecs, W):
    u_vecs = np.ascontiguousarray(u_vecs, dtype=np.float32)
    W = np.ascontiguousarray(W, dtype=np.float32)
    wot = np.ascontiguousarray(W.reshape(NC, DC, ID).transpose(2, 0, 1))  # [d', n, d]
    wg = np.ascontiguousarray(W.reshape(NC, DC, ID).transpose(1, 0, 2))   # [d, n, d']
    wot_hi, wot_lo = _split16(wot)
    # pre-tile wot to [128, NCH, NC, DC]
    wot_hi = np.ascontiguousarray(wot_hi.reshape(NCH, 128, NC, DC).transpose(1, 0, 2, 3))
    wot_lo = np.ascontiguousarray(wot_lo.reshape(NCH, 128, NC, DC).transpose(1, 0, 2, 3))
    wg_hi, wg_lo = _split16(wg)

    u4 = u_vecs.reshape(NCORES, BPC, IN, ID)
    un_hi, un_lo = _split16(u4)
    # u_nat pre-tiled: [BPC, IN, ID] -> [BPC, 128, NT, ID]
    def tile_nat(x):
        return np.ascontiguousarray(
            x.reshape(NCORES, BPC, NT, 128, ID).transpose(0, 1, 3, 2, 4))
    # u_t pre-tiled: [BPC, ID, IN] -> [BPC, 128, NCH, IN]
    def tile_t(x):
        xt = x.transpose(0, 1, 3, 2)  # [NCORES, BPC, ID, IN]
        return np.ascontiguousarray(
            xt.reshape(NCORES, BPC, NCH, 128, IN).transpose(0, 1, 3, 2, 4))
    un_hi_t, un_lo_t = tile_nat(un_hi), tile_nat(un_lo)
    ut_hi_t, ut_lo_t = tile_t(un_hi), tile_t(un_lo)
    return [
        {"un_hi": un_hi_t[k], "un_lo": un_lo_t[k],
         "ut_hi": ut_hi_t[k], "ut_lo": ut_lo_t[k],
         "wot_hi": wot_hi, "wot_lo": wot_lo,
         "wg_hi": wg_hi, "wg_lo": wg_lo}
        for k in range(NCORES)
    ]


def kernel(u_vecs: np.ndarray, W: np.ndarray) -> np.ndarray:
    from concourse.bass_utils import run_bass_kernel_spmd

    nc = get_program()
    in_maps = make_in_maps(u_vecs, W)
    res = run_bass_kernel_spmd(nc, in_maps, list(range(NCORES))).results
    # per-core out: [DC, NC, BPC] -> [BPC, NC, DC]; stack cores -> [B, NC, DC]
    parts = [res[k]["out"].transpose(2, 1, 0) for k in range(NCORES)]
    return np.ascontiguousarray(np.concatenate(parts, axis=0))
